# revision 15
# baseline (speedup 1.0000x reference)
"""Trainium2 Bass kernel for nn_AAGF_704374636718 (nms_detection).

Pure data parallel: B=8 images across 8 NeuronCores, one image per core.

Per-core pipeline (one image, C=256 channels as 2 partition-tiles of 128):
  Stage A (ROI path):
    - DMA each feat c-tile to SBUF, GPSIMD ap_gather the 8x8 integer patches
      around every anchor (bilinear support; edge-clamped duplicate rows/cols
      make uniform fractional weights exact).
    - DVE separable bilinear interp (y then x) with per-anchor weight maps
      (broadcast stride-0 APs) -> R[tensor][ct] (128, 128*49).
    - ROI logits via PE: logit_diff = u.R_rgb + v.R_tir (the channel dot
      commutes with interp), 8 anchors per PSUM accumulation group.
    - ACT sigmoid -> per-anchor-pixel attention; PE ones-matmul broadcasts it
      across partitions; DVE blends fused = R_tir + (R_rgb - R_tir) * sigma
      into the merge-source tile.
  Stage B (global fuse + merge), per 16-row chunk:
    - PE: logit_diff into PSUM (4 x 512 sub-chunks), ACT sigmoid,
      PE ones-matmul broadcast across 128 partitions
    - DVE blend: out = b + (a - b) * sigma  -> merge-source tile
    - GPSIMD ap_gather merge: final[pix] = concat(fused_roi, chunk)[inv_map]
      (inv_map encodes the sequential last-writer-wins patch scatter, computed
      host-side from anchor metadata)
    - DMA chunk to DRAM output.

Host preprocessing is limited to O(B*N) anchor metadata: gather index tables,
bilinear weight tables, and the scatter winner map.
"""

import contextlib

import numpy as np

import concourse.bacc as bacc
import concourse.bass as bass
import concourse.tile as tile
from concourse import mybir
from concourse.bass_utils import run_bass_kernel_spmd

ROI = 7
H = W = 128
C = 256
CT = 2          # channel tiles of 128
P = 128         # partitions
N = 128         # anchors
HW = H * W
NPIX = N * ROI * ROI            # 6272 fused-roi columns
CHUNK_ROWS = 16
CHUNK_PIX = CHUNK_ROWS * W      # 2048
NCHUNK = H // CHUNK_ROWS        # 8
SRC_W = NPIX + CHUNK_PIX        # merge-gather source width: 8320
F32 = mybir.dt.float32
I16 = mybir.dt.int16


def _wrap_idx(idx_flat):
    """Lay out a flat int index list for ap_gather: idx[i] must sit at
    partition (i % 16) (replicated across the 8 Q7 groups), column i // 16."""
    n = idx_flat.shape[0]
    assert n % 16 == 0
    w = idx_flat.reshape(n // 16, 16).T.astype(np.int16)   # (16, n//16)
    return np.tile(w, (8, 1))                              # (128, n//16)


def host_prep_image(a_rgb, a_tir):
    """Per-image aux tensors from anchor metadata (all O(N))."""
    aux = {}
    hyhx = []
    for name, a in (("rgb", a_rgb), ("tir", a_tir)):
        ax = a[:, 0].astype(np.float32)
        ay = a[:, 1].astype(np.float32)
        xi = np.floor(ax).astype(np.int32) - 3
        yi = np.floor(ay).astype(np.int32) - 3
        cols = np.clip(xi[:, None] + np.arange(8), 0, W - 1)        # (N,8)
        rows = np.clip(yi[:, None] + np.arange(8), 0, H - 1)        # (N,8)
        gidx = (rows[:, :, None] * W + cols[:, None, :]).reshape(-1)  # (N*64,)
        aux[f"gidx_{name}"] = _wrap_idx(gidx)
        hyhx.append((np.float32(1.0) - (ay - np.floor(ay))).astype(np.float32))
        hyhx.append((np.float32(1.0) - (ax - np.floor(ax))).astype(np.float32))
    # wtab: (128, 512) f32 = [hy_rgb | hx_rgb | hy_tir | hx_tir], replicated rows
    aux["wtab"] = np.tile(np.concatenate(hyhx).astype(np.float32)[None, :], (P, 1))

    # sequential scatter winner map (rgb anchors drive positions)
    ax = a_rgb[:, 0].astype(np.float32)
    ay = a_rgb[:, 1].astype(np.float32)
    x0 = np.clip(np.trunc(ax - np.float32(3.5)).astype(np.int32), 0, W - ROI)
    y0 = np.clip(np.trunc(ay - np.float32(3.5)).astype(np.int32), 0, H - ROI)
    paint = np.full(HW, -1, np.int32)
    for n in range(N):
        for q in range(ROI):
            base = (y0[n] + q) * W + x0[n]
            paint[base:base + ROI] = n * 49 + q * 7 + np.arange(7)
    # per-chunk merge indices into [fused_roi (6272) | chunk_global (2048)]
    minv_cols = []
    for c in range(NCHUNK):
        seg = paint[c * CHUNK_PIX:(c + 1) * CHUNK_PIX]
        inv = np.where(seg >= 0, seg, NPIX + np.arange(CHUNK_PIX))
        minv_cols.append(_wrap_idx(inv))                   # (128, 128)
    aux["minv"] = np.concatenate(minv_cols, axis=1)        # (128, 1024)
    return aux


def host_prep_weights(w_global, b_global, w_att, b_att):
    u_g = (w_global[0] - w_global[1]).astype(np.float32)   # (512,)
    u_a = (w_att[0] - w_att[1]).astype(np.float32)
    # uw: (128, 8): [g_rgb0, g_rgb1, g_tir0, g_tir1, a_rgb0, a_rgb1, a_tir0, a_tir1]
    uw = np.stack([u_g[0:128], u_g[128:256], u_g[256:384], u_g[384:512],
                   u_a[0:128], u_a[128:256], u_a[256:384], u_a[384:512]],
                  axis=1).astype(np.float32)
    c_g = float(np.float32(b_global[0]) - np.float32(b_global[1]))
    c_a = float(np.float32(b_att[0]) - np.float32(b_att[1]))
    return uw, c_g, c_a


def _bcast_w(wt, col_off, n_off, n_cnt, q_cnt, x_cnt):
    """AP over the wtab tile: (128p, n_cnt, q_cnt, x_cnt) reading per-anchor
    weight wtab[p, col_off + n_off + n], broadcast over q/x via 0-strides.
    The trailing broadcast dims must come after the real n dim, so broadcast
    (p, n, q, x) with q/x stride-0, then let the caller transpose if needed."""
    s = wt[:, col_off + n_off:col_off + n_off + n_cnt]
    return s.to_broadcast([P, n_cnt, q_cnt, x_cnt])


def build_program(c_g, c_a):
    nc = bacc.Bacc("TRN2", target_bir_lowering=False, debug=False, num_devices=8)

    fr = nc.dram_tensor("feat_rgb", [C, H, W], F32, kind="ExternalInput")
    ft = nc.dram_tensor("feat_tir", [C, H, W], F32, kind="ExternalInput")
    gidx_r = nc.dram_tensor("gidx_rgb", [P, 512], I16, kind="ExternalInput")
    gidx_t = nc.dram_tensor("gidx_tir", [P, 512], I16, kind="ExternalInput")
    minv = nc.dram_tensor("minv", [P, NCHUNK * 128], I16, kind="ExternalInput")
    wtab = nc.dram_tensor("wtab", [P, 512], F32, kind="ExternalInput")
    uw = nc.dram_tensor("uw", [P, 8], F32, kind="ExternalInput")
    out = nc.dram_tensor("out", [C, H, W], F32, kind="ExternalOutput")

    fr_ap = fr.ap().rearrange("c h w -> c (h w)")
    ft_ap = ft.ap().rearrange("c h w -> c (h w)")
    out_ap = out.ap().rearrange("c h w -> c (h w)")

    with tile.TileContext(nc) as tc, contextlib.ExitStack() as octx:
        persist = octx.enter_context(tc.tile_pool(name="persist", bufs=1))
        gidx_r_sb = persist.tile([P, 512], I16)
        nc.sync.dma_start(out=gidx_r_sb[:], in_=gidx_r.ap())
        gidx_t_sb = persist.tile([P, 512], I16)
        nc.sync.dma_start(out=gidx_t_sb[:], in_=gidx_t.ap())
        wtab_sb = persist.tile([P, 512], F32)
        nc.sync.dma_start(out=wtab_sb[:], in_=wtab.ap())
        uw_sb = persist.tile([P, 8], F32)
        nc.sync.dma_start(out=uw_sb[:], in_=uw.ap())
        ones1 = persist.tile([1, P], F32)
        nc.vector.memset(ones1[:], 1.0)

        # ---------------- Stage A: ROI path ----------------
        R = {}
        with tc.tile_pool(name="rpool", bufs=1, side="right") as rpool:
            with tc.tile_pool(name="gpool", bufs=1, side="right") as gpool, \
                 tc.tile_pool(name="tmpa", bufs=1, side="right") as tmpa, \
                 tc.tile_pool(name="featp", bufs=1, side="right") as featp:
                for ttype, fap, gsb in (("tir", ft_ap, gidx_t_sb),
                                        ("rgb", fr_ap, gidx_r_sb)):
                    wy_off = 0 if ttype == "rgb" else 256
                    wx_off = 128 if ttype == "rgb" else 384
                    for ct in range(CT):
                        fbuf = featp.tile([P, HW], F32, tag="fbuf")
                        nc.sync.dma_start(out=fbuf[:],
                                          in_=fap[ct * P:(ct + 1) * P, :])
                        r = rpool.tile([P, NPIX], F32, tag=f"R{ttype}{ct}",
                                       name=f"R{ttype}{ct}")
                        R[(ttype, ct)] = r
                        r4 = r[:].rearrange("p (n q x) -> p n q x",
                                            n=N, q=ROI, x=ROI)
                        # gather + interp in half-anchor batches of 64
                        for h in range(2):
                            g = gpool.tile([P, 64 * 64], F32, tag="G", bufs=2)
                            nc.gpsimd.ap_gather(
                                out_ap=g[:], in_ap=fbuf[:],
                                idxs_ap=gsb[:, h * 256:(h + 1) * 256],
                                channels=P, num_elems=HW, d=1, num_idxs=64 * 64)
                            g4 = g[:].rearrange("p (n q x) -> p n q x",
                                                n=64, q=8, x=8)
                            for qt in range(4):   # 16 anchors per interp chunk
                                ns = slice(qt * 16, (qt + 1) * 16)
                                nabs = h * 64 + qt * 16
                                t1 = tmpa.tile([P, 16, 7, 8], F32, tag="t1")
                                nc.vector.tensor_tensor(
                                    out=t1[:], in0=g4[:, ns, 0:7, :],
                                    in1=g4[:, ns, 1:8, :],
                                    op=mybir.AluOpType.subtract)
                                nc.vector.tensor_tensor(
                                    out=t1[:], in0=t1[:],
                                    in1=_bcast_w(wtab_sb, wy_off, nabs, 16, 7, 8),
                                    op=mybir.AluOpType.mult)
                                yb = tmpa.tile([P, 16, 7, 8], F32, tag="yb")
                                nc.vector.tensor_tensor(
                                    out=yb[:], in0=t1[:], in1=g4[:, ns, 1:8, :],
                                    op=mybir.AluOpType.add)
                                t2 = tmpa.tile([P, 16, 7, 7], F32, tag="t1")
                                nc.vector.tensor_tensor(
                                    out=t2[:], in0=yb[:, :, :, 0:7],
                                    in1=yb[:, :, :, 1:8],
                                    op=mybir.AluOpType.subtract)
                                nc.vector.tensor_tensor(
                                    out=t2[:], in0=t2[:],
                                    in1=_bcast_w(wtab_sb, wx_off, nabs, 16, 7, 7),
                                    op=mybir.AluOpType.mult)
                                nc.vector.tensor_tensor(
                                    out=r4[:, h * 64 + qt * 16:h * 64 + (qt + 1) * 16,
                                           :, :],
                                    in0=t2[:], in1=yb[:, :, :, 1:8],
                                    op=mybir.AluOpType.add)

            # S pool opens once the feat/gather pools are closed; it lives on
            # the LEFT side until the end (fused-roi values + per-chunk
            # global workspace for the merge gather).
            spool = octx.enter_context(tc.tile_pool(name="spool", bufs=1))
            S = [spool.tile([P, SRC_W], F32, tag=f"S{ct}", name=f"S{ct}")
                 for ct in range(CT)]

            # ROI logits (8 anchors / 392 cols per PSUM group), sigmoid, fuse
            with tc.tile_pool(name="fusep", bufs=1, side="right") as fusep, \
                 tc.tile_pool(name="pspa", bufs=1, space="PSUM") as pspa:
                for k in range(16):
                    cs = slice(k * 8 * 49, (k + 1) * 8 * 49)
                    lp = pspa.tile([1, 392], F32, tag="lp", bufs=2)
                    mm = [("rgb", 0, 4), ("rgb", 1, 5),
                          ("tir", 0, 6), ("tir", 1, 7)]
                    for i, (tt, ct, uc) in enumerate(mm):
                        nc.tensor.matmul(
                            out=lp[:], lhsT=uw_sb[:, uc:uc + 1],
                            rhs=R[(tt, ct)][:, cs],
                            start=(i == 0), stop=(i == 3))
                    sig = fusep.tile([1, 392], F32, tag="sig", bufs=2)
                    nc.scalar.activation(
                        out=sig[:], in_=lp[:],
                        func=mybir.ActivationFunctionType.Sigmoid, bias=c_a)
                    sb = pspa.tile([P, 392], F32, tag="sbro", bufs=2)
                    nc.tensor.matmul(out=sb[:], lhsT=ones1[:], rhs=sig[:],
                                     start=True, stop=True)
                    for ct in range(CT):
                        d = fusep.tile([P, 392], F32, tag="fuse", bufs=2)
                        nc.vector.tensor_tensor(
                            out=d[:], in0=R[("rgb", ct)][:, cs],
                            in1=R[("tir", ct)][:, cs],
                            op=mybir.AluOpType.subtract)
                        nc.vector.tensor_tensor(
                            out=d[:], in0=d[:], in1=sb[:],
                            op=mybir.AluOpType.mult)
                        nc.vector.tensor_tensor(
                            out=S[ct][:, cs], in0=d[:],
                            in1=R[("tir", ct)][:, cs],
                            op=mybir.AluOpType.add)

        # ---------------- Stage B: global fuse + merge ----------------
        with tc.tile_pool(name="bpool", bufs=1, side="right") as bpool, \
             tc.tile_pool(name="pspb", bufs=1, space="PSUM") as pspb:
            minv_sb = bpool.tile([P, NCHUNK * 128], I16)
            nc.sync.dma_start(out=minv_sb[:], in_=minv.ap())
            for c in range(NCHUNK):
                pix = slice(c * CHUNK_PIX, (c + 1) * CHUNK_PIX)
                ab = {}
                for tt, fap in (("rgb", fr_ap), ("tir", ft_ap)):
                    for ct in range(CT):
                        t = bpool.tile([P, CHUNK_PIX], F32,
                                       tag=f"ch{tt}{ct}", bufs=2,
                                       name=f"ch{tt}{ct}")
                        nc.sync.dma_start(out=t[:],
                                          in_=fap[ct * P:(ct + 1) * P, pix])
                        ab[(tt, ct)] = t
                sgg = bpool.tile([1, CHUNK_PIX], F32, tag="sgg", bufs=2)
                for j in range(4):
                    js = slice(j * 512, (j + 1) * 512)
                    lg = pspb.tile([1, 512], F32, tag="lg", bufs=2)
                    mm = [("rgb", 0, 0), ("rgb", 1, 1),
                          ("tir", 0, 2), ("tir", 1, 3)]
                    for i, (tt, ct, uc) in enumerate(mm):
                        nc.tensor.matmul(
                            out=lg[:], lhsT=uw_sb[:, uc:uc + 1],
                            rhs=ab[(tt, ct)][:, js],
                            start=(i == 0), stop=(i == 3))
                    nc.scalar.activation(
                        out=sgg[:, js], in_=lg[:],
                        func=mybir.ActivationFunctionType.Sigmoid, bias=c_g)
                dbc = pspb.tile([P, CHUNK_PIX], F32, tag="dbc", bufs=1)
                for j in range(4):
                    nc.tensor.matmul(
                        out=dbc[:, j * 512:(j + 1) * 512],
                        lhsT=ones1[:], rhs=sgg[:, j * 512:(j + 1) * 512],
                        start=True, stop=True)
                for ct in range(CT):
                    t = bpool.tile([P, CHUNK_PIX], F32, tag="gbl", bufs=2)
                    nc.vector.tensor_tensor(
                        out=t[:], in0=ab[("rgb", ct)][:], in1=ab[("tir", ct)][:],
                        op=mybir.AluOpType.subtract)
                    nc.vector.tensor_tensor(
                        out=t[:], in0=t[:], in1=dbc[:], op=mybir.AluOpType.mult)
                    nc.vector.tensor_tensor(
                        out=S[ct][:, NPIX:], in0=t[:], in1=ab[("tir", ct)][:],
                        op=mybir.AluOpType.add)
                    final = bpool.tile([P, CHUNK_PIX], F32,
                                       tag=f"fin{ct}", bufs=2, name=f"fin{ct}")
                    nc.gpsimd.ap_gather(
                        out_ap=final[:], in_ap=S[ct][:],
                        idxs_ap=minv_sb[:, c * 128:(c + 1) * 128],
                        channels=P, num_elems=SRC_W, d=1, num_idxs=CHUNK_PIX)
                    nc.sync.dma_start(
                        out=out_ap[ct * P:(ct + 1) * P, pix], in_=final[:])

    nc.compile()
    return nc


_CACHE = {}


def kernel(feat_rgb, feat_tir, anchors_rgb_with_conf, anchors_tir_with_conf,
           w_global, b_global, w_att, b_att):
    feat_rgb = np.asarray(feat_rgb, dtype=np.float32)
    feat_tir = np.asarray(feat_tir, dtype=np.float32)
    a_rgb = np.asarray(anchors_rgb_with_conf, dtype=np.float32)
    a_tir = np.asarray(anchors_tir_with_conf, dtype=np.float32)
    w_global = np.asarray(w_global, dtype=np.float32)
    b_global = np.asarray(b_global, dtype=np.float32)
    w_att = np.asarray(w_att, dtype=np.float32)
    b_att = np.asarray(b_att, dtype=np.float32)

    B = feat_rgb.shape[0]
    assert B == 8

    uw, c_g, c_a = host_prep_weights(w_global, b_global, w_att, b_att)
    key = (c_g, c_a)
    if key not in _CACHE:
        _CACHE[key] = build_program(c_g, c_a)
    nc = _CACHE[key]

    in_maps = []
    for b in range(B):
        aux = host_prep_image(a_rgb[b], a_tir[b])
        in_maps.append({
            "feat_rgb": np.ascontiguousarray(feat_rgb[b]),
            "feat_tir": np.ascontiguousarray(feat_tir[b]),
            "gidx_rgb": aux["gidx_rgb"],
            "gidx_tir": aux["gidx_tir"],
            "minv": aux["minv"],
            "wtab": aux["wtab"],
            "uw": uw,
        })

    res = run_bass_kernel_spmd(nc, in_maps, core_ids=list(range(8)))
    global LAST_RUN
    LAST_RUN = res
    outs = [res.results[b]["out"] for b in range(B)]
    return np.stack(outs).astype(np.float32)


LAST_RUN = None


def time_kernel_ns(feat_rgb, feat_tir, anchors_rgb_with_conf,
                   anchors_tir_with_conf, w_global, b_global, w_att, b_att,
                   inner_iters=8, outer_iters=3):
    """Best-effort HW execution time: run the compiled NEFF `inner_iters`
    times inside one jitted call (serialized through the donated output
    buffers so XLA cannot dedupe), amortizing the axon dispatch overhead.
    Returns ns per NEFF execution (min over outer_iters)."""
    import time as _time
    import jax
    import jax.numpy as jnp
    from jax.sharding import Mesh, PartitionSpec
    from jax.experimental.shard_map import shard_map
    from concourse import bass2jax

    feat_rgb = np.asarray(feat_rgb, dtype=np.float32)
    feat_tir = np.asarray(feat_tir, dtype=np.float32)
    a_rgb = np.asarray(anchors_rgb_with_conf, dtype=np.float32)
    a_tir = np.asarray(anchors_tir_with_conf, dtype=np.float32)
    uw, c_g, c_a = host_prep_weights(
        np.asarray(w_global, np.float32), np.asarray(b_global, np.float32),
        np.asarray(w_att, np.float32), np.asarray(b_att, np.float32))
    key = (c_g, c_a)
    if key not in _CACHE:
        _CACHE[key] = build_program(c_g, c_a)
    nc = _CACHE[key]
    bass2jax.install_neuronx_cc_hook()

    in_names = []
    out_names = []
    out_avals = []
    partition_name = nc.partition_id_tensor.name if nc.partition_id_tensor else None
    for alloc in nc.m.functions[0].allocations:
        import concourse.mybir as mybir_
        if not isinstance(alloc, mybir_.MemoryLocationSet):
            continue
        name = alloc.memorylocations[0].name
        if alloc.kind == "ExternalInput":
            if name != partition_name:
                in_names.append(name)
        elif alloc.kind == "ExternalOutput":
            out_names.append(name)
            out_avals.append(jax.core.ShapedArray(
                tuple(alloc.tensor_shape), mybir.dt.np(alloc.dtype)))
    n_params = len(in_names)
    all_names = list(in_names) + list(out_names)
    if partition_name is not None:
        all_names.append(partition_name)

    def _body(*args):
        operands = list(args)
        if partition_name is not None:
            operands.append(bass2jax.partition_id_tensor())
        outs = bass2jax._bass_exec_p.bind(
            *operands, out_avals=tuple(out_avals), in_names=tuple(all_names),
            out_names=tuple(out_names), lowering_input_output_aliases=(),
            sim_require_finite=True, sim_require_nnan=True, nc=nc)
        return tuple(outs)

    del inner_iters  # the compile hook allows one bass_exec per module
    _loop = _body

    in_maps = []
    B = feat_rgb.shape[0]
    for b in range(B):
        aux = host_prep_image(a_rgb[b], a_tir[b])
        in_maps.append({
            "feat_rgb": np.ascontiguousarray(feat_rgb[b]),
            "feat_tir": np.ascontiguousarray(feat_tir[b]),
            "gidx_rgb": aux["gidx_rgb"], "gidx_tir": aux["gidx_tir"],
            "minv": aux["minv"], "wtab": aux["wtab"], "uw": uw,
        })

    devices = jax.devices()[:B]
    mesh = Mesh(np.asarray(devices), ("core",))
    n_outs = len(out_names)
    sharded = jax.jit(
        shard_map(_loop, mesh=mesh,
                  in_specs=(PartitionSpec("core"),) * (n_params + n_outs),
                  out_specs=(PartitionSpec("core"),) * n_outs,
                  check_rep=False),
        keep_unused=True)
    concat_in = [np.concatenate([np.asarray(in_maps[c][nm]) for c in range(B)],
                                axis=0) for nm in in_names]
    concat_zeros = [np.zeros((B * a.shape[0], *a.shape[1:]), a.dtype)
                    for a in out_avals]
    from jax.sharding import NamedSharding
    shard = NamedSharding(mesh, PartitionSpec("core"))
    dev_in = [jax.device_put(x, shard) for x in concat_in]
    dev_zeros = [jax.device_put(z, shard) for z in concat_zeros]
    # dispatch-overhead baseline: a trivial jitted op on the same mesh
    tiny = jax.jit(shard_map(lambda x: (x * 1.0,), mesh=mesh,
                             in_specs=(PartitionSpec("core"),),
                             out_specs=(PartitionSpec("core"),),
                             check_rep=False))
    jax.block_until_ready(tiny(dev_in[2]))
    base = None
    for _ in range(outer_iters + 2):
        t0 = _time.perf_counter()
        jax.block_until_ready(tiny(dev_in[2]))
        dt = _time.perf_counter() - t0
        base = dt if base is None else min(base, dt)
    # warmup (compiles)
    jax.block_until_ready(sharded(*dev_in, *dev_zeros))
    best = None
    for _ in range(outer_iters):
        t0 = _time.perf_counter()
        jax.block_until_ready(sharded(*dev_in, *dev_zeros))
        dt = _time.perf_counter() - t0
        best = dt if best is None else min(best, dt)
    print(f"  (raw call {best*1e6:.0f} us, dispatch baseline {base*1e6:.0f} us)")
    return max(best - base, 0.0) * 1e9


if __name__ == "__main__":
    import reference
    inputs = reference.setup_inputs()
    actual = kernel(**{k: np.asarray(v) for k, v in inputs.items()})
    expected = np.asarray(reference.reference(**inputs))
    err = np.abs(actual - expected).max()
    print("abs err:", err, "rel:", err / np.abs(expected).max())


# revision 18
# speedup vs baseline: 1.3156x; 1.3156x over previous
"""Trainium2 Bass kernel for nn_AAGF_704374636718 (nms_detection).

Pure data parallel: B=8 images across 8 NeuronCores, one image per core.

Per-core pipeline (one image, C=256 channels as 2 partition-tiles of 128):
  Stage A (ROI path):
    - DMA each feat c-tile to SBUF, GPSIMD ap_gather the 8x8 integer patches
      around every anchor (bilinear support; edge-clamped duplicate rows/cols
      make uniform fractional weights exact).
    - DVE separable bilinear interp (y then x) with per-anchor weight maps
      (broadcast stride-0 APs) -> R[tensor][ct] (128, 128*49).
    - ROI logits via PE: logit_diff = u.R_rgb + v.R_tir (the channel dot
      commutes with interp), 8 anchors per PSUM accumulation group.
    - ACT sigmoid -> per-anchor-pixel attention; PE ones-matmul broadcasts it
      across partitions; DVE blends fused = R_tir + (R_rgb - R_tir) * sigma
      into the merge-source tile.
  Stage B (global fuse + merge), per 16-row chunk:
    - PE: logit_diff into PSUM (4 x 512 sub-chunks), ACT sigmoid,
      PE ones-matmul broadcast across 128 partitions
    - DVE blend: out = b + (a - b) * sigma  -> merge-source tile
    - GPSIMD ap_gather merge: final[pix] = concat(fused_roi, chunk)[inv_map]
      (inv_map encodes the sequential last-writer-wins patch scatter, computed
      host-side from anchor metadata)
    - DMA chunk to DRAM output.

Host preprocessing is limited to O(B*N) anchor metadata: gather index tables,
bilinear weight tables, and the scatter winner map.
"""

import contextlib

import numpy as np

import concourse.bacc as bacc
import concourse.bass as bass
import concourse.tile as tile
from concourse import mybir
from concourse.bass_utils import run_bass_kernel_spmd

ROI = 7
H = W = 128
C = 256
CT = 2          # channel tiles of 128
P = 128         # partitions
N = 128         # anchors
HW = H * W
NPIX = N * ROI * ROI            # 6272 fused-roi columns
CHUNK_ROWS = 16
CHUNK_PIX = CHUNK_ROWS * W      # 2048
NCHUNK = H // CHUNK_ROWS        # 8
SRC_W = NPIX + CHUNK_PIX        # merge-gather source width: 8320
F32 = mybir.dt.float32
I16 = mybir.dt.int16


def _wrap_idx(idx_flat):
    """Lay out a flat int index list for ap_gather: idx[i] must sit at
    partition (i % 16) (replicated across the 8 Q7 groups), column i // 16."""
    n = idx_flat.shape[0]
    assert n % 16 == 0
    w = idx_flat.reshape(n // 16, 16).T.astype(np.int16)   # (16, n//16)
    return np.tile(w, (8, 1))                              # (128, n//16)


def host_prep_image(a_rgb, a_tir):
    """Per-image aux tensors from anchor metadata (all O(N))."""
    aux = {}
    hyhx = []
    for name, a in (("rgb", a_rgb), ("tir", a_tir)):
        ax = a[:, 0].astype(np.float32)
        ay = a[:, 1].astype(np.float32)
        xi = np.floor(ax).astype(np.int32) - 3
        yi = np.floor(ay).astype(np.int32) - 3
        cols = np.clip(xi[:, None] + np.arange(8), 0, W - 1)        # (N,8)
        rows = np.clip(yi[:, None] + np.arange(8), 0, H - 1)        # (N,8)
        gidx = (rows[:, :, None] * W + cols[:, None, :]).reshape(-1)  # (N*64,)
        aux[f"gidx_{name}"] = _wrap_idx(gidx)
        hyhx.append((np.float32(1.0) - (ay - np.floor(ay))).astype(np.float32))
        hyhx.append((np.float32(1.0) - (ax - np.floor(ax))).astype(np.float32))
    # wtab: (128, 512) f32 = [hy_rgb | hx_rgb | hy_tir | hx_tir], replicated rows
    aux["wtab"] = np.tile(np.concatenate(hyhx).astype(np.float32)[None, :], (P, 1))

    # sequential scatter winner map (rgb anchors drive positions)
    ax = a_rgb[:, 0].astype(np.float32)
    ay = a_rgb[:, 1].astype(np.float32)
    x0 = np.clip(np.trunc(ax - np.float32(3.5)).astype(np.int32), 0, W - ROI)
    y0 = np.clip(np.trunc(ay - np.float32(3.5)).astype(np.int32), 0, H - ROI)
    paint = np.full(HW, -1, np.int32)
    for n in range(N):
        for q in range(ROI):
            base = (y0[n] + q) * W + x0[n]
            paint[base:base + ROI] = n * 49 + q * 7 + np.arange(7)
    # per-chunk merge indices into [fused_roi (6272) | chunk_global (2048)]
    minv_cols = []
    for c in range(NCHUNK):
        seg = paint[c * CHUNK_PIX:(c + 1) * CHUNK_PIX]
        inv = np.where(seg >= 0, seg, NPIX + np.arange(CHUNK_PIX))
        minv_cols.append(_wrap_idx(inv))                   # (128, 128)
    aux["minv"] = np.concatenate(minv_cols, axis=1)        # (128, 1024)
    return aux


def host_prep_weights(w_global, b_global, w_att, b_att):
    u_g = (w_global[0] - w_global[1]).astype(np.float32)   # (512,)
    u_a = (w_att[0] - w_att[1]).astype(np.float32)
    # uw: (128, 8): [g_rgb0, g_rgb1, g_tir0, g_tir1, a_rgb0, a_rgb1, a_tir0, a_tir1]
    uw = np.stack([u_g[0:128], u_g[128:256], u_g[256:384], u_g[384:512],
                   u_a[0:128], u_a[128:256], u_a[256:384], u_a[384:512]],
                  axis=1).astype(np.float32)
    c_g = float(np.float32(b_global[0]) - np.float32(b_global[1]))
    c_a = float(np.float32(b_att[0]) - np.float32(b_att[1]))
    return uw, c_g, c_a


def _bcast_w(wt, col_off, n_off, n_cnt, q_cnt, x_cnt):
    """AP over the wtab tile: (128p, n_cnt, q_cnt, x_cnt) reading per-anchor
    weight wtab[p, col_off + n_off + n], broadcast over q/x via 0-strides.
    The trailing broadcast dims must come after the real n dim, so broadcast
    (p, n, q, x) with q/x stride-0, then let the caller transpose if needed."""
    s = wt[:, col_off + n_off:col_off + n_off + n_cnt]
    return s.to_broadcast([P, n_cnt, q_cnt, x_cnt])


def build_program(c_g, c_a):
    nc = bacc.Bacc("TRN2", target_bir_lowering=False, debug=False, num_devices=8)

    fr = nc.dram_tensor("feat_rgb", [C, H, W], F32, kind="ExternalInput")
    ft = nc.dram_tensor("feat_tir", [C, H, W], F32, kind="ExternalInput")
    gidx_r = nc.dram_tensor("gidx_rgb", [P, 512], I16, kind="ExternalInput")
    gidx_t = nc.dram_tensor("gidx_tir", [P, 512], I16, kind="ExternalInput")
    minv = nc.dram_tensor("minv", [P, NCHUNK * 128], I16, kind="ExternalInput")
    wtab = nc.dram_tensor("wtab", [P, 512], F32, kind="ExternalInput")
    uw = nc.dram_tensor("uw", [P, 8], F32, kind="ExternalInput")
    out = nc.dram_tensor("out", [C, H, W], F32, kind="ExternalOutput")

    fr_ap = fr.ap().rearrange("c h w -> c (h w)")
    ft_ap = ft.ap().rearrange("c h w -> c (h w)")
    out_ap = out.ap().rearrange("c h w -> c (h w)")

    with tile.TileContext(nc) as tc, contextlib.ExitStack() as octx:
        persist = octx.enter_context(tc.tile_pool(name="persist", bufs=1))
        gidx_r_sb = persist.tile([P, 512], I16)
        nc.sync.dma_start(out=gidx_r_sb[:], in_=gidx_r.ap())
        gidx_t_sb = persist.tile([P, 512], I16)
        nc.sync.dma_start(out=gidx_t_sb[:], in_=gidx_t.ap())
        wtab_sb = persist.tile([P, 512], F32)
        nc.sync.dma_start(out=wtab_sb[:], in_=wtab.ap())
        uw_sb = persist.tile([P, 8], F32)
        nc.sync.dma_start(out=uw_sb[:], in_=uw.ap())
        ones1 = persist.tile([1, P], F32)
        nc.vector.memset(ones1[:], 1.0)

        # ---------------- Stage A: ROI path ----------------
        R = {}
        with tc.tile_pool(name="rpool", bufs=1, side="right") as rpool:
            with tc.tile_pool(name="gpool", bufs=1, side="right") as gpool, \
                 tc.tile_pool(name="tmpa", bufs=1, side="right") as tmpa, \
                 tc.tile_pool(name="featp", bufs=1, side="right") as featp:
                for ttype, fap, gsb in (("tir", ft_ap, gidx_t_sb),
                                        ("rgb", fr_ap, gidx_r_sb)):
                    wy_off = 0 if ttype == "rgb" else 256
                    wx_off = 128 if ttype == "rgb" else 384
                    for ct in range(CT):
                        fbuf = featp.tile([P, HW], F32, tag="fbuf")
                        nc.sync.dma_start(out=fbuf[:],
                                          in_=fap[ct * P:(ct + 1) * P, :])
                        r = rpool.tile([P, NPIX], F32, tag=f"R{ttype}{ct}",
                                       name=f"R{ttype}{ct}")
                        R[(ttype, ct)] = r
                        r4 = r[:].rearrange("p (n q x) -> p n q x",
                                            n=N, q=ROI, x=ROI)
                        # gather + interp in half-anchor batches of 64
                        for h in range(2):
                            g = gpool.tile([P, 64 * 64], F32, tag="G", bufs=2)
                            nc.gpsimd.ap_gather(
                                out_ap=g[:], in_ap=fbuf[:],
                                idxs_ap=gsb[:, h * 256:(h + 1) * 256],
                                channels=P, num_elems=HW, d=1, num_idxs=64 * 64)
                            g4 = g[:].rearrange("p (n q x) -> p n q x",
                                                n=64, q=8, x=8)
                            for qt in range(4):   # 16 anchors per interp chunk
                                ns = slice(qt * 16, (qt + 1) * 16)
                                nabs = h * 64 + qt * 16
                                t1 = tmpa.tile([P, 16, 7, 8], F32, tag="t1")
                                nc.vector.tensor_tensor(
                                    out=t1[:], in0=g4[:, ns, 0:7, :],
                                    in1=g4[:, ns, 1:8, :],
                                    op=mybir.AluOpType.subtract)
                                nc.vector.tensor_tensor(
                                    out=t1[:], in0=t1[:],
                                    in1=_bcast_w(wtab_sb, wy_off, nabs, 16, 7, 8),
                                    op=mybir.AluOpType.mult)
                                yb = tmpa.tile([P, 16, 7, 8], F32, tag="yb")
                                nc.vector.tensor_tensor(
                                    out=yb[:], in0=t1[:], in1=g4[:, ns, 1:8, :],
                                    op=mybir.AluOpType.add)
                                t2 = tmpa.tile([P, 16, 7, 7], F32, tag="t1")
                                nc.vector.tensor_tensor(
                                    out=t2[:], in0=yb[:, :, :, 0:7],
                                    in1=yb[:, :, :, 1:8],
                                    op=mybir.AluOpType.subtract)
                                nc.vector.tensor_tensor(
                                    out=t2[:], in0=t2[:],
                                    in1=_bcast_w(wtab_sb, wx_off, nabs, 16, 7, 7),
                                    op=mybir.AluOpType.mult)
                                nc.vector.tensor_tensor(
                                    out=r4[:, h * 64 + qt * 16:h * 64 + (qt + 1) * 16,
                                           :, :],
                                    in0=t2[:], in1=yb[:, :, :, 1:8],
                                    op=mybir.AluOpType.add)

            # S pool opens once the feat/gather pools are closed; it lives on
            # the LEFT side until the end (fused-roi values + per-chunk
            # global workspace for the merge gather).
            spool = octx.enter_context(tc.tile_pool(name="spool", bufs=1))
            S = [spool.tile([P, SRC_W], F32, tag=f"S{ct}", name=f"S{ct}")
                 for ct in range(CT)]

            # ROI logits (8 anchors / 392 cols per PSUM group), sigmoid, fuse
            with tc.tile_pool(name="fusep", bufs=1, side="right") as fusep, \
                 tc.tile_pool(name="pspa", bufs=1, space="PSUM") as pspa:
                for k in range(16):
                    cs = slice(k * 8 * 49, (k + 1) * 8 * 49)
                    lp = pspa.tile([1, 392], F32, tag="lp", bufs=2)
                    mm = [("rgb", 0, 4), ("rgb", 1, 5),
                          ("tir", 0, 6), ("tir", 1, 7)]
                    for i, (tt, ct, uc) in enumerate(mm):
                        nc.tensor.matmul(
                            out=lp[:], lhsT=uw_sb[:, uc:uc + 1],
                            rhs=R[(tt, ct)][:, cs],
                            start=(i == 0), stop=(i == 3))
                    sig = fusep.tile([1, 392], F32, tag="sig", bufs=2)
                    nc.scalar.activation(
                        out=sig[:], in_=lp[:],
                        func=mybir.ActivationFunctionType.Sigmoid, bias=c_a)
                    sb = pspa.tile([P, 392], F32, tag="sbro", bufs=2)
                    nc.tensor.matmul(out=sb[:], lhsT=ones1[:], rhs=sig[:],
                                     start=True, stop=True)
                    for ct in range(CT):
                        d = fusep.tile([P, 392], F32, tag="fuse", bufs=2)
                        nc.vector.tensor_tensor(
                            out=d[:], in0=R[("rgb", ct)][:, cs],
                            in1=R[("tir", ct)][:, cs],
                            op=mybir.AluOpType.subtract)
                        nc.vector.tensor_tensor(
                            out=d[:], in0=d[:], in1=sb[:],
                            op=mybir.AluOpType.mult)
                        nc.vector.tensor_tensor(
                            out=S[ct][:, cs], in0=d[:],
                            in1=R[("tir", ct)][:, cs],
                            op=mybir.AluOpType.add)

        # ---------------- Stage B: global fuse + merge ----------------
        with tc.tile_pool(name="bpool", bufs=1, side="right") as bpool, \
             tc.tile_pool(name="pspb", bufs=1, space="PSUM") as pspb:
            minv_sb = bpool.tile([P, NCHUNK * 128], I16)
            nc.sync.dma_start(out=minv_sb[:], in_=minv.ap())
            for c in range(NCHUNK):
                pix = slice(c * CHUNK_PIX, (c + 1) * CHUNK_PIX)
                ab = {}
                for tt, fap in (("rgb", fr_ap), ("tir", ft_ap)):
                    for ct in range(CT):
                        t = bpool.tile([P, CHUNK_PIX], F32,
                                       tag=f"ch{tt}{ct}", bufs=2,
                                       name=f"ch{tt}{ct}")
                        nc.sync.dma_start(out=t[:],
                                          in_=fap[ct * P:(ct + 1) * P, pix])
                        ab[(tt, ct)] = t
                sgg = bpool.tile([1, CHUNK_PIX], F32, tag="sgg", bufs=2)
                for j in range(4):
                    js = slice(j * 512, (j + 1) * 512)
                    lg = pspb.tile([1, 512], F32, tag="lg", bufs=2)
                    mm = [("rgb", 0, 0), ("rgb", 1, 1),
                          ("tir", 0, 2), ("tir", 1, 3)]
                    for i, (tt, ct, uc) in enumerate(mm):
                        nc.tensor.matmul(
                            out=lg[:], lhsT=uw_sb[:, uc:uc + 1],
                            rhs=ab[(tt, ct)][:, js],
                            start=(i == 0), stop=(i == 3))
                    nc.scalar.activation(
                        out=sgg[:, js], in_=lg[:],
                        func=mybir.ActivationFunctionType.Sigmoid, bias=c_g)
                dbc = pspb.tile([P, CHUNK_PIX], F32, tag="dbc", bufs=1)
                for j in range(4):
                    nc.tensor.matmul(
                        out=dbc[:, j * 512:(j + 1) * 512],
                        lhsT=ones1[:], rhs=sgg[:, j * 512:(j + 1) * 512],
                        start=True, stop=True)
                for ct in range(CT):
                    t = bpool.tile([P, CHUNK_PIX], F32, tag="gbl", bufs=2)
                    nc.vector.tensor_tensor(
                        out=t[:], in0=ab[("rgb", ct)][:], in1=ab[("tir", ct)][:],
                        op=mybir.AluOpType.subtract)
                    nc.vector.tensor_tensor(
                        out=t[:], in0=t[:], in1=dbc[:], op=mybir.AluOpType.mult)
                    nc.vector.tensor_tensor(
                        out=S[ct][:, NPIX:], in0=t[:], in1=ab[("tir", ct)][:],
                        op=mybir.AluOpType.add)
                    final = bpool.tile([P, CHUNK_PIX], F32,
                                       tag=f"fin{ct}", bufs=2, name=f"fin{ct}")
                    nc.gpsimd.ap_gather(
                        out_ap=final[:], in_ap=S[ct][:],
                        idxs_ap=minv_sb[:, c * 128:(c + 1) * 128],
                        channels=P, num_elems=SRC_W, d=1, num_idxs=CHUNK_PIX)
                    nc.sync.dma_start(
                        out=out_ap[ct * P:(ct + 1) * P, pix], in_=final[:])

    nc.compile()
    return nc


_CACHE = {}


def kernel(feat_rgb, feat_tir, anchors_rgb_with_conf, anchors_tir_with_conf,
           w_global, b_global, w_att, b_att):
    feat_rgb = np.asarray(feat_rgb, dtype=np.float32)
    feat_tir = np.asarray(feat_tir, dtype=np.float32)
    a_rgb = np.asarray(anchors_rgb_with_conf, dtype=np.float32)
    a_tir = np.asarray(anchors_tir_with_conf, dtype=np.float32)
    w_global = np.asarray(w_global, dtype=np.float32)
    b_global = np.asarray(b_global, dtype=np.float32)
    w_att = np.asarray(w_att, dtype=np.float32)
    b_att = np.asarray(b_att, dtype=np.float32)

    B = feat_rgb.shape[0]
    assert B == 8

    uw, c_g, c_a = host_prep_weights(w_global, b_global, w_att, b_att)
    key = (c_g, c_a)
    if key not in _CACHE:
        _CACHE[key] = build_program(c_g, c_a)
    nc = _CACHE[key]

    in_maps = []
    for b in range(B):
        aux = host_prep_image(a_rgb[b], a_tir[b])
        in_maps.append({
            "feat_rgb": np.ascontiguousarray(feat_rgb[b]),
            "feat_tir": np.ascontiguousarray(feat_tir[b]),
            "gidx_rgb": aux["gidx_rgb"],
            "gidx_tir": aux["gidx_tir"],
            "minv": aux["minv"],
            "wtab": aux["wtab"],
            "uw": uw,
        })

    res = run_bass_kernel_spmd(nc, in_maps, core_ids=list(range(8)))
    global LAST_RUN
    LAST_RUN = res
    outs = [res.results[b]["out"] for b in range(B)]
    return np.stack(outs).astype(np.float32)


LAST_RUN = None


def time_kernel_ns(feat_rgb, feat_tir, anchors_rgb_with_conf,
                   anchors_tir_with_conf, w_global, b_global, w_att, b_att,
                   inner_iters=8, outer_iters=3):
    """Best-effort HW execution time: run the compiled NEFF `inner_iters`
    times inside one jitted call (serialized through the donated output
    buffers so XLA cannot dedupe), amortizing the axon dispatch overhead.
    Returns ns per NEFF execution (min over outer_iters)."""
    import time as _time
    import jax
    import jax.numpy as jnp
    from jax.sharding import Mesh, PartitionSpec
    from jax.experimental.shard_map import shard_map
    from concourse import bass2jax

    feat_rgb = np.asarray(feat_rgb, dtype=np.float32)
    feat_tir = np.asarray(feat_tir, dtype=np.float32)
    a_rgb = np.asarray(anchors_rgb_with_conf, dtype=np.float32)
    a_tir = np.asarray(anchors_tir_with_conf, dtype=np.float32)
    uw, c_g, c_a = host_prep_weights(
        np.asarray(w_global, np.float32), np.asarray(b_global, np.float32),
        np.asarray(w_att, np.float32), np.asarray(b_att, np.float32))
    key = (c_g, c_a)
    if key not in _CACHE:
        _CACHE[key] = build_program(c_g, c_a)
    nc = _CACHE[key]
    bass2jax.install_neuronx_cc_hook()

    in_names = []
    out_names = []
    out_avals = []
    partition_name = nc.partition_id_tensor.name if nc.partition_id_tensor else None
    for alloc in nc.m.functions[0].allocations:
        import concourse.mybir as mybir_
        if not isinstance(alloc, mybir_.MemoryLocationSet):
            continue
        name = alloc.memorylocations[0].name
        if alloc.kind == "ExternalInput":
            if name != partition_name:
                in_names.append(name)
        elif alloc.kind == "ExternalOutput":
            out_names.append(name)
            out_avals.append(jax.core.ShapedArray(
                tuple(alloc.tensor_shape), mybir.dt.np(alloc.dtype)))
    n_params = len(in_names)
    all_names = list(in_names) + list(out_names)
    if partition_name is not None:
        all_names.append(partition_name)

    def _body(*args):
        operands = list(args)
        if partition_name is not None:
            operands.append(bass2jax.partition_id_tensor())
        outs = bass2jax._bass_exec_p.bind(
            *operands, out_avals=tuple(out_avals), in_names=tuple(all_names),
            out_names=tuple(out_names), lowering_input_output_aliases=(),
            sim_require_finite=True, sim_require_nnan=True, nc=nc)
        return tuple(outs)

    del inner_iters  # the compile hook allows one bass_exec per module
    _loop = _body

    in_maps = []
    B = feat_rgb.shape[0]
    for b in range(B):
        aux = host_prep_image(a_rgb[b], a_tir[b])
        in_maps.append({
            "feat_rgb": np.ascontiguousarray(feat_rgb[b]),
            "feat_tir": np.ascontiguousarray(feat_tir[b]),
            "gidx_rgb": aux["gidx_rgb"], "gidx_tir": aux["gidx_tir"],
            "minv": aux["minv"], "wtab": aux["wtab"], "uw": uw,
        })

    devices = jax.devices()[:B]
    mesh = Mesh(np.asarray(devices), ("core",))
    n_outs = len(out_names)
    sharded = jax.jit(
        shard_map(_loop, mesh=mesh,
                  in_specs=(PartitionSpec("core"),) * (n_params + n_outs),
                  out_specs=(PartitionSpec("core"),) * n_outs,
                  check_rep=False),
        keep_unused=True)
    concat_in = [np.concatenate([np.asarray(in_maps[c][nm]) for c in range(B)],
                                axis=0) for nm in in_names]
    concat_zeros = [np.zeros((B * a.shape[0], *a.shape[1:]), a.dtype)
                    for a in out_avals]
    from jax.sharding import NamedSharding
    shard = NamedSharding(mesh, PartitionSpec("core"))
    dev_in = [jax.device_put(x, shard) for x in concat_in]
    dev_zeros = [jax.device_put(z, shard) for z in concat_zeros]
    # warmup (compiles)
    jax.block_until_ready(sharded(*dev_in, *dev_zeros))

    def run_k(k):
        best = None
        for _ in range(outer_iters):
            t0 = _time.perf_counter()
            outs = None
            for _ in range(k):
                outs = sharded(*dev_in, *dev_zeros)
            jax.block_until_ready(outs)
            dt = _time.perf_counter() - t0
            best = dt if best is None else min(best, dt)
        return best

    t1 = run_k(1)
    t9 = run_k(9)
    per_exec = (t9 - t1) / 8
    print(f"  (1 call {t1*1e6:.0f} us, 9 calls {t9*1e6:.0f} us)")
    return max(per_exec, 0.0) * 1e9


if __name__ == "__main__":
    import reference
    inputs = reference.setup_inputs()
    actual = kernel(**{k: np.asarray(v) for k, v in inputs.items()})
    expected = np.asarray(reference.reference(**inputs))
    err = np.abs(actual - expected).max()
    print("abs err:", err, "rel:", err / np.abs(expected).max())


# revision 21
# speedup vs baseline: 1.4170x; 1.0771x over previous
"""Trainium2 Bass kernel for nn_AAGF_704374636718 (nms_detection).

Pure data parallel: B=8 images across 8 NeuronCores, one image per core.

Per-core pipeline (one image, C=256 channels as 2 partition-tiles of 128):
  Stage A (ROI path):
    - DMA each feat c-tile to SBUF, GPSIMD ap_gather the 8x8 integer patches
      around every anchor (bilinear support; edge-clamped duplicate rows/cols
      make uniform fractional weights exact).
    - DVE separable bilinear interp (y then x) with per-anchor weight maps
      (broadcast stride-0 APs) -> R[tensor][ct] (128, 128*49).
    - ROI logits via PE: logit_diff = u.R_rgb + v.R_tir (the channel dot
      commutes with interp), 8 anchors per PSUM accumulation group.
    - ACT sigmoid -> per-anchor-pixel attention; PE ones-matmul broadcasts it
      across partitions; DVE blends fused = R_tir + (R_rgb - R_tir) * sigma
      into the merge-source tile.
  Stage B (global fuse + merge), per 16-row chunk:
    - PE: logit_diff into PSUM (4 x 512 sub-chunks), ACT sigmoid,
      PE ones-matmul broadcast across 128 partitions
    - DVE blend: out = b + (a - b) * sigma  -> merge-source tile
    - GPSIMD ap_gather merge: final[pix] = concat(fused_roi, chunk)[inv_map]
      (inv_map encodes the sequential last-writer-wins patch scatter, computed
      host-side from anchor metadata)
    - DMA chunk to DRAM output.

Host preprocessing is limited to O(B*N) anchor metadata: gather index tables,
bilinear weight tables, and the scatter winner map.
"""

import contextlib
import os

import numpy as np

_ABLATE_GATHER = bool(os.environ.get("KERNEL_ABLATE_GATHER"))

import concourse.bacc as bacc
import concourse.bass as bass
import concourse.tile as tile
from concourse import mybir
from concourse.bass_utils import run_bass_kernel_spmd

ROI = 7
H = W = 128
C = 256
CT = 2          # channel tiles of 128
P = 128         # partitions
N = 128         # anchors
HW = H * W
NPIX = N * ROI * ROI            # 6272 fused-roi columns
CHUNK_ROWS = 16
CHUNK_PIX = CHUNK_ROWS * W      # 2048
NCHUNK = H // CHUNK_ROWS        # 8
SRC_W = NPIX + CHUNK_PIX        # merge-gather source width: 8320
F32 = mybir.dt.float32
I16 = mybir.dt.int16


def _wrap_idx(idx_flat):
    """Lay out a flat int index list for ap_gather: idx[i] must sit at
    partition (i % 16) (replicated across the 8 Q7 groups), column i // 16."""
    n = idx_flat.shape[0]
    assert n % 16 == 0
    w = idx_flat.reshape(n // 16, 16).T.astype(np.int16)   # (16, n//16)
    return np.tile(w, (8, 1))                              # (128, n//16)


def host_prep_image(a_rgb, a_tir):
    """Per-image aux tensors from anchor metadata (all O(N))."""
    aux = {}
    hyhx = []
    for name, a in (("rgb", a_rgb), ("tir", a_tir)):
        ax = a[:, 0].astype(np.float32)
        ay = a[:, 1].astype(np.float32)
        xi = np.floor(ax).astype(np.int32) - 3
        yi = np.floor(ay).astype(np.int32) - 3
        cols = np.clip(xi[:, None] + np.arange(8), 0, W - 1)        # (N,8)
        rows = np.clip(yi[:, None] + np.arange(8), 0, H - 1)        # (N,8)
        gidx = (rows[:, :, None] * W + cols[:, None, :]).reshape(-1)  # (N*64,)
        aux[f"gidx_{name}"] = _wrap_idx(gidx)
        hyhx.append((np.float32(1.0) - (ay - np.floor(ay))).astype(np.float32))
        hyhx.append((np.float32(1.0) - (ax - np.floor(ax))).astype(np.float32))
    # wtab: (128, 512) f32 = [hy_rgb | hx_rgb | hy_tir | hx_tir], replicated rows
    aux["wtab"] = np.tile(np.concatenate(hyhx).astype(np.float32)[None, :], (P, 1))

    # sequential scatter winner map (rgb anchors drive positions)
    ax = a_rgb[:, 0].astype(np.float32)
    ay = a_rgb[:, 1].astype(np.float32)
    x0 = np.clip(np.trunc(ax - np.float32(3.5)).astype(np.int32), 0, W - ROI)
    y0 = np.clip(np.trunc(ay - np.float32(3.5)).astype(np.int32), 0, H - ROI)
    paint = np.full(HW, -1, np.int32)
    for n in range(N):
        for q in range(ROI):
            base = (y0[n] + q) * W + x0[n]
            paint[base:base + ROI] = n * 49 + q * 7 + np.arange(7)
    # per-chunk merge indices into [fused_roi (6272) | chunk_global (2048)]
    minv_cols = []
    for c in range(NCHUNK):
        seg = paint[c * CHUNK_PIX:(c + 1) * CHUNK_PIX]
        inv = np.where(seg >= 0, seg, NPIX + np.arange(CHUNK_PIX))
        minv_cols.append(_wrap_idx(inv))                   # (128, 128)
    aux["minv"] = np.concatenate(minv_cols, axis=1)        # (128, 1024)
    return aux


def host_prep_weights(w_global, b_global, w_att, b_att):
    u_g = (w_global[0] - w_global[1]).astype(np.float32)   # (512,)
    u_a = (w_att[0] - w_att[1]).astype(np.float32)
    # uw: (128, 8): [g_rgb0, g_rgb1, g_tir0, g_tir1, a_rgb0, a_rgb1, a_tir0, a_tir1]
    uw = np.stack([u_g[0:128], u_g[128:256], u_g[256:384], u_g[384:512],
                   u_a[0:128], u_a[128:256], u_a[256:384], u_a[384:512]],
                  axis=1).astype(np.float32)
    c_g = float(np.float32(b_global[0]) - np.float32(b_global[1]))
    c_a = float(np.float32(b_att[0]) - np.float32(b_att[1]))
    return uw, c_g, c_a


def _bcast_w(wt, col_off, n_off, n_cnt, q_cnt, x_cnt):
    """AP over the wtab tile: (128p, n_cnt, q_cnt, x_cnt) reading per-anchor
    weight wtab[p, col_off + n_off + n], broadcast over q/x via 0-strides.
    The trailing broadcast dims must come after the real n dim, so broadcast
    (p, n, q, x) with q/x stride-0, then let the caller transpose if needed."""
    s = wt[:, col_off + n_off:col_off + n_off + n_cnt]
    return s.to_broadcast([P, n_cnt, q_cnt, x_cnt])


def build_program(c_g, c_a):
    nc = bacc.Bacc("TRN2", target_bir_lowering=False, debug=False, num_devices=8)

    fr = nc.dram_tensor("feat_rgb", [C, H, W], F32, kind="ExternalInput")
    ft = nc.dram_tensor("feat_tir", [C, H, W], F32, kind="ExternalInput")
    gidx_r = nc.dram_tensor("gidx_rgb", [P, 512], I16, kind="ExternalInput")
    gidx_t = nc.dram_tensor("gidx_tir", [P, 512], I16, kind="ExternalInput")
    minv = nc.dram_tensor("minv", [P, NCHUNK * 128], I16, kind="ExternalInput")
    wtab = nc.dram_tensor("wtab", [P, 512], F32, kind="ExternalInput")
    uw = nc.dram_tensor("uw", [P, 8], F32, kind="ExternalInput")
    out = nc.dram_tensor("out", [C, H, W], F32, kind="ExternalOutput")

    fr_ap = fr.ap().rearrange("c h w -> c (h w)")
    ft_ap = ft.ap().rearrange("c h w -> c (h w)")
    out_ap = out.ap().rearrange("c h w -> c (h w)")

    with tile.TileContext(nc) as tc, contextlib.ExitStack() as octx:
        persist = octx.enter_context(tc.tile_pool(name="persist", bufs=1))
        gidx_r_sb = persist.tile([P, 512], I16)
        nc.sync.dma_start(out=gidx_r_sb[:], in_=gidx_r.ap())
        gidx_t_sb = persist.tile([P, 512], I16)
        nc.sync.dma_start(out=gidx_t_sb[:], in_=gidx_t.ap())
        wtab_sb = persist.tile([P, 512], F32)
        nc.sync.dma_start(out=wtab_sb[:], in_=wtab.ap())
        uw_sb = persist.tile([P, 8], F32)
        nc.sync.dma_start(out=uw_sb[:], in_=uw.ap())
        ones1 = persist.tile([1, P], F32)
        nc.vector.memset(ones1[:], 1.0)

        # ---------------- Stage A: ROI path ----------------
        R = {}
        with tc.tile_pool(name="rpool", bufs=1, side="right") as rpool:
            with tc.tile_pool(name="gpool", bufs=1, side="right") as gpool, \
                 tc.tile_pool(name="tmpa", bufs=1, side="right") as tmpa, \
                 tc.tile_pool(name="featp", bufs=1, side="right") as featp:
                for ttype, fap, gsb in (("tir", ft_ap, gidx_t_sb),
                                        ("rgb", fr_ap, gidx_r_sb)):
                    wy_off = 0 if ttype == "rgb" else 256
                    wx_off = 128 if ttype == "rgb" else 384
                    for ct in range(CT):
                        fbuf = featp.tile([P, HW], F32, tag="fbuf")
                        nc.sync.dma_start(out=fbuf[:],
                                          in_=fap[ct * P:(ct + 1) * P, :])
                        r = rpool.tile([P, NPIX], F32, tag=f"R{ttype}{ct}",
                                       name=f"R{ttype}{ct}")
                        R[(ttype, ct)] = r
                        r4 = r[:].rearrange("p (n q x) -> p n q x",
                                            n=N, q=ROI, x=ROI)
                        # gather + interp in half-anchor batches of 64
                        for h in range(2):
                            g = gpool.tile([P, 64 * 64], F32, tag="G", bufs=2)
                            if _ABLATE_GATHER:
                                nc.vector.tensor_copy(g[:], fbuf[:, :64 * 64])
                            else:
                                nc.gpsimd.ap_gather(
                                    out_ap=g[:], in_ap=fbuf[:],
                                    idxs_ap=gsb[:, h * 256:(h + 1) * 256],
                                    channels=P, num_elems=HW, d=1,
                                    num_idxs=64 * 64)
                            g4 = g[:].rearrange("p (n q x) -> p n q x",
                                                n=64, q=8, x=8)
                            for qt in range(4):   # 16 anchors per interp chunk
                                ns = slice(qt * 16, (qt + 1) * 16)
                                nabs = h * 64 + qt * 16
                                t1 = tmpa.tile([P, 16, 7, 8], F32, tag="t1")
                                nc.vector.tensor_tensor(
                                    out=t1[:], in0=g4[:, ns, 0:7, :],
                                    in1=g4[:, ns, 1:8, :],
                                    op=mybir.AluOpType.subtract)
                                nc.vector.tensor_tensor(
                                    out=t1[:], in0=t1[:],
                                    in1=_bcast_w(wtab_sb, wy_off, nabs, 16, 7, 8),
                                    op=mybir.AluOpType.mult)
                                yb = tmpa.tile([P, 16, 7, 8], F32, tag="yb")
                                nc.vector.tensor_tensor(
                                    out=yb[:], in0=t1[:], in1=g4[:, ns, 1:8, :],
                                    op=mybir.AluOpType.add)
                                t2 = tmpa.tile([P, 16, 7, 7], F32, tag="t1")
                                nc.vector.tensor_tensor(
                                    out=t2[:], in0=yb[:, :, :, 0:7],
                                    in1=yb[:, :, :, 1:8],
                                    op=mybir.AluOpType.subtract)
                                nc.vector.tensor_tensor(
                                    out=t2[:], in0=t2[:],
                                    in1=_bcast_w(wtab_sb, wx_off, nabs, 16, 7, 7),
                                    op=mybir.AluOpType.mult)
                                nc.vector.tensor_tensor(
                                    out=r4[:, h * 64 + qt * 16:h * 64 + (qt + 1) * 16,
                                           :, :],
                                    in0=t2[:], in1=yb[:, :, :, 1:8],
                                    op=mybir.AluOpType.add)

            # S pool opens once the feat/gather pools are closed; it lives on
            # the LEFT side until the end (fused-roi values + per-chunk
            # global workspace for the merge gather).
            spool = octx.enter_context(tc.tile_pool(name="spool", bufs=1))
            S = [spool.tile([P, SRC_W], F32, tag=f"S{ct}", name=f"S{ct}")
                 for ct in range(CT)]

            # ROI logits (8 anchors / 392 cols per PSUM group), sigmoid, fuse
            with tc.tile_pool(name="fusep", bufs=1, side="right") as fusep, \
                 tc.tile_pool(name="pspa", bufs=1, space="PSUM") as pspa:
                for k in range(16):
                    cs = slice(k * 8 * 49, (k + 1) * 8 * 49)
                    lp = pspa.tile([1, 392], F32, tag="lp", bufs=2)
                    mm = [("rgb", 0, 4), ("rgb", 1, 5),
                          ("tir", 0, 6), ("tir", 1, 7)]
                    for i, (tt, ct, uc) in enumerate(mm):
                        nc.tensor.matmul(
                            out=lp[:], lhsT=uw_sb[:, uc:uc + 1],
                            rhs=R[(tt, ct)][:, cs],
                            start=(i == 0), stop=(i == 3))
                    sig = fusep.tile([1, 392], F32, tag="sig", bufs=2)
                    nc.scalar.activation(
                        out=sig[:], in_=lp[:],
                        func=mybir.ActivationFunctionType.Sigmoid, bias=c_a)
                    sb = pspa.tile([P, 392], F32, tag="sbro", bufs=2)
                    nc.tensor.matmul(out=sb[:], lhsT=ones1[:], rhs=sig[:],
                                     start=True, stop=True)
                    for ct in range(CT):
                        d = fusep.tile([P, 392], F32, tag="fuse", bufs=2)
                        nc.vector.tensor_tensor(
                            out=d[:], in0=R[("rgb", ct)][:, cs],
                            in1=R[("tir", ct)][:, cs],
                            op=mybir.AluOpType.subtract)
                        nc.vector.tensor_tensor(
                            out=d[:], in0=d[:], in1=sb[:],
                            op=mybir.AluOpType.mult)
                        nc.vector.tensor_tensor(
                            out=S[ct][:, cs], in0=d[:],
                            in1=R[("tir", ct)][:, cs],
                            op=mybir.AluOpType.add)

        # ---------------- Stage B: global fuse + merge ----------------
        with tc.tile_pool(name="bpool", bufs=1, side="right") as bpool, \
             tc.tile_pool(name="pspb", bufs=1, space="PSUM") as pspb:
            minv_sb = bpool.tile([P, NCHUNK * 128], I16)
            nc.sync.dma_start(out=minv_sb[:], in_=minv.ap())
            for c in range(NCHUNK):
                pix = slice(c * CHUNK_PIX, (c + 1) * CHUNK_PIX)
                ab = {}
                for tt, fap in (("rgb", fr_ap), ("tir", ft_ap)):
                    for ct in range(CT):
                        t = bpool.tile([P, CHUNK_PIX], F32,
                                       tag=f"ch{tt}{ct}", bufs=2,
                                       name=f"ch{tt}{ct}")
                        nc.sync.dma_start(out=t[:],
                                          in_=fap[ct * P:(ct + 1) * P, pix])
                        ab[(tt, ct)] = t
                sgg = bpool.tile([1, CHUNK_PIX], F32, tag="sgg", bufs=2)
                for j in range(4):
                    js = slice(j * 512, (j + 1) * 512)
                    lg = pspb.tile([1, 512], F32, tag="lg", bufs=2)
                    mm = [("rgb", 0, 0), ("rgb", 1, 1),
                          ("tir", 0, 2), ("tir", 1, 3)]
                    for i, (tt, ct, uc) in enumerate(mm):
                        nc.tensor.matmul(
                            out=lg[:], lhsT=uw_sb[:, uc:uc + 1],
                            rhs=ab[(tt, ct)][:, js],
                            start=(i == 0), stop=(i == 3))
                    nc.scalar.activation(
                        out=sgg[:, js], in_=lg[:],
                        func=mybir.ActivationFunctionType.Sigmoid, bias=c_g)
                dbc = pspb.tile([P, CHUNK_PIX], F32, tag="dbc", bufs=1)
                for j in range(4):
                    nc.tensor.matmul(
                        out=dbc[:, j * 512:(j + 1) * 512],
                        lhsT=ones1[:], rhs=sgg[:, j * 512:(j + 1) * 512],
                        start=True, stop=True)
                for ct in range(CT):
                    t = bpool.tile([P, CHUNK_PIX], F32, tag="gbl", bufs=2)
                    nc.vector.tensor_tensor(
                        out=t[:], in0=ab[("rgb", ct)][:], in1=ab[("tir", ct)][:],
                        op=mybir.AluOpType.subtract)
                    nc.vector.tensor_tensor(
                        out=t[:], in0=t[:], in1=dbc[:], op=mybir.AluOpType.mult)
                    nc.vector.tensor_tensor(
                        out=S[ct][:, NPIX:], in0=t[:], in1=ab[("tir", ct)][:],
                        op=mybir.AluOpType.add)
                    final = bpool.tile([P, CHUNK_PIX], F32,
                                       tag=f"fin{ct}", bufs=2, name=f"fin{ct}")
                    if _ABLATE_GATHER:
                        nc.vector.tensor_copy(final[:], S[ct][:, NPIX:])
                    else:
                        nc.gpsimd.ap_gather(
                            out_ap=final[:], in_ap=S[ct][:],
                            idxs_ap=minv_sb[:, c * 128:(c + 1) * 128],
                            channels=P, num_elems=SRC_W, d=1,
                            num_idxs=CHUNK_PIX)
                    nc.sync.dma_start(
                        out=out_ap[ct * P:(ct + 1) * P, pix], in_=final[:])

    nc.compile()
    return nc


_CACHE = {}


def kernel(feat_rgb, feat_tir, anchors_rgb_with_conf, anchors_tir_with_conf,
           w_global, b_global, w_att, b_att):
    feat_rgb = np.asarray(feat_rgb, dtype=np.float32)
    feat_tir = np.asarray(feat_tir, dtype=np.float32)
    a_rgb = np.asarray(anchors_rgb_with_conf, dtype=np.float32)
    a_tir = np.asarray(anchors_tir_with_conf, dtype=np.float32)
    w_global = np.asarray(w_global, dtype=np.float32)
    b_global = np.asarray(b_global, dtype=np.float32)
    w_att = np.asarray(w_att, dtype=np.float32)
    b_att = np.asarray(b_att, dtype=np.float32)

    B = feat_rgb.shape[0]
    assert B == 8

    uw, c_g, c_a = host_prep_weights(w_global, b_global, w_att, b_att)
    key = (c_g, c_a)
    if key not in _CACHE:
        _CACHE[key] = build_program(c_g, c_a)
    nc = _CACHE[key]

    in_maps = []
    for b in range(B):
        aux = host_prep_image(a_rgb[b], a_tir[b])
        in_maps.append({
            "feat_rgb": np.ascontiguousarray(feat_rgb[b]),
            "feat_tir": np.ascontiguousarray(feat_tir[b]),
            "gidx_rgb": aux["gidx_rgb"],
            "gidx_tir": aux["gidx_tir"],
            "minv": aux["minv"],
            "wtab": aux["wtab"],
            "uw": uw,
        })

    res = run_bass_kernel_spmd(nc, in_maps, core_ids=list(range(8)))
    global LAST_RUN
    LAST_RUN = res
    outs = [res.results[b]["out"] for b in range(B)]
    return np.stack(outs).astype(np.float32)


LAST_RUN = None


def time_kernel_ns(feat_rgb, feat_tir, anchors_rgb_with_conf,
                   anchors_tir_with_conf, w_global, b_global, w_att, b_att,
                   inner_iters=8, outer_iters=3):
    """Best-effort HW execution time: run the compiled NEFF `inner_iters`
    times inside one jitted call (serialized through the donated output
    buffers so XLA cannot dedupe), amortizing the axon dispatch overhead.
    Returns ns per NEFF execution (min over outer_iters)."""
    import time as _time
    import jax
    import jax.numpy as jnp
    from jax.sharding import Mesh, PartitionSpec
    from jax.experimental.shard_map import shard_map
    from concourse import bass2jax

    feat_rgb = np.asarray(feat_rgb, dtype=np.float32)
    feat_tir = np.asarray(feat_tir, dtype=np.float32)
    a_rgb = np.asarray(anchors_rgb_with_conf, dtype=np.float32)
    a_tir = np.asarray(anchors_tir_with_conf, dtype=np.float32)
    uw, c_g, c_a = host_prep_weights(
        np.asarray(w_global, np.float32), np.asarray(b_global, np.float32),
        np.asarray(w_att, np.float32), np.asarray(b_att, np.float32))
    key = (c_g, c_a)
    if key not in _CACHE:
        _CACHE[key] = build_program(c_g, c_a)
    nc = _CACHE[key]
    bass2jax.install_neuronx_cc_hook()

    in_names = []
    out_names = []
    out_avals = []
    partition_name = nc.partition_id_tensor.name if nc.partition_id_tensor else None
    for alloc in nc.m.functions[0].allocations:
        import concourse.mybir as mybir_
        if not isinstance(alloc, mybir_.MemoryLocationSet):
            continue
        name = alloc.memorylocations[0].name
        if alloc.kind == "ExternalInput":
            if name != partition_name:
                in_names.append(name)
        elif alloc.kind == "ExternalOutput":
            out_names.append(name)
            out_avals.append(jax.core.ShapedArray(
                tuple(alloc.tensor_shape), mybir.dt.np(alloc.dtype)))
    n_params = len(in_names)
    all_names = list(in_names) + list(out_names)
    if partition_name is not None:
        all_names.append(partition_name)

    def _body(*args):
        operands = list(args)
        if partition_name is not None:
            operands.append(bass2jax.partition_id_tensor())
        outs = bass2jax._bass_exec_p.bind(
            *operands, out_avals=tuple(out_avals), in_names=tuple(all_names),
            out_names=tuple(out_names), lowering_input_output_aliases=(),
            sim_require_finite=True, sim_require_nnan=True, nc=nc)
        return tuple(outs)

    del inner_iters  # the compile hook allows one bass_exec per module
    _loop = _body

    in_maps = []
    B = feat_rgb.shape[0]
    for b in range(B):
        aux = host_prep_image(a_rgb[b], a_tir[b])
        in_maps.append({
            "feat_rgb": np.ascontiguousarray(feat_rgb[b]),
            "feat_tir": np.ascontiguousarray(feat_tir[b]),
            "gidx_rgb": aux["gidx_rgb"], "gidx_tir": aux["gidx_tir"],
            "minv": aux["minv"], "wtab": aux["wtab"], "uw": uw,
        })

    devices = jax.devices()[:B]
    mesh = Mesh(np.asarray(devices), ("core",))
    n_outs = len(out_names)
    sharded = jax.jit(
        shard_map(_loop, mesh=mesh,
                  in_specs=(PartitionSpec("core"),) * (n_params + n_outs),
                  out_specs=(PartitionSpec("core"),) * n_outs,
                  check_rep=False),
        keep_unused=True)
    concat_in = [np.concatenate([np.asarray(in_maps[c][nm]) for c in range(B)],
                                axis=0) for nm in in_names]
    concat_zeros = [np.zeros((B * a.shape[0], *a.shape[1:]), a.dtype)
                    for a in out_avals]
    from jax.sharding import NamedSharding
    shard = NamedSharding(mesh, PartitionSpec("core"))
    dev_in = [jax.device_put(x, shard) for x in concat_in]
    dev_zeros = [jax.device_put(z, shard) for z in concat_zeros]
    # warmup (compiles)
    jax.block_until_ready(sharded(*dev_in, *dev_zeros))

    def run_k(k):
        best = None
        for _ in range(outer_iters):
            t0 = _time.perf_counter()
            outs = None
            for _ in range(k):
                outs = sharded(*dev_in, *dev_zeros)
            jax.block_until_ready(outs)
            dt = _time.perf_counter() - t0
            best = dt if best is None else min(best, dt)
        return best

    t1 = run_k(1)
    t9 = run_k(9)
    per_exec = (t9 - t1) / 8
    print(f"  (1 call {t1*1e6:.0f} us, 9 calls {t9*1e6:.0f} us)")
    return max(per_exec, 0.0) * 1e9


if __name__ == "__main__":
    import reference
    inputs = reference.setup_inputs()
    actual = kernel(**{k: np.asarray(v) for k, v in inputs.items()})
    expected = np.asarray(reference.reference(**inputs))
    err = np.abs(actual - expected).max()
    print("abs err:", err, "rel:", err / np.abs(expected).max())


# revision 25
# speedup vs baseline: 2.1276x; 1.5014x over previous
"""Trainium2 Bass kernel for nn_AAGF_704374636718 (nms_detection).

Pure data parallel: B=8 images across 8 NeuronCores, one image per core.

Per-core pipeline (one image, C=256 channels as 2 partition-tiles of 128):
  Stage A (ROI path):
    - DMA each feat c-tile to SBUF, GPSIMD ap_gather the 8x8 integer patches
      around every anchor (bilinear support; edge-clamped duplicate rows/cols
      make uniform fractional weights exact).
    - DVE separable bilinear interp (y then x) with per-anchor weight maps
      (broadcast stride-0 APs) -> R[tensor][ct] (128, 128*49).
    - ROI logits via PE: logit_diff = u.R_rgb + v.R_tir (the channel dot
      commutes with interp), 8 anchors per PSUM accumulation group.
    - ACT sigmoid -> per-anchor-pixel attention; PE ones-matmul broadcasts it
      across partitions; DVE blends fused = R_tir + (R_rgb - R_tir) * sigma
      into the merge-source tile.
  Stage B (global fuse + merge), per 16-row chunk:
    - PE: logit_diff into PSUM (4 x 512 sub-chunks), ACT sigmoid,
      PE ones-matmul broadcast across 128 partitions
    - DVE blend: out = b + (a - b) * sigma  -> merge-source tile
    - GPSIMD ap_gather merge: final[pix] = concat(fused_roi, chunk)[inv_map]
      (inv_map encodes the sequential last-writer-wins patch scatter, computed
      host-side from anchor metadata)
    - DMA chunk to DRAM output.

Host preprocessing is limited to O(B*N) anchor metadata: gather index tables,
bilinear weight tables, and the scatter winner map.
"""

import contextlib
import os

import numpy as np

_ABLATE_GATHER = bool(os.environ.get("KERNEL_ABLATE_GATHER"))

import concourse.bacc as bacc
import concourse.bass as bass
import concourse.tile as tile
from concourse import mybir
from concourse.bass_utils import run_bass_kernel_spmd

ROI = 7
H = W = 128
C = 256
CT = 2          # channel tiles of 128
P = 128         # partitions
N = 128         # anchors
HW = H * W
NPIX = N * ROI * ROI            # 6272 fused-roi columns
CHUNK_ROWS = 16
CHUNK_PIX = CHUNK_ROWS * W      # 2048
NCHUNK = H // CHUNK_ROWS        # 8
SRC_W = NPIX + CHUNK_PIX        # merge-gather source width: 8320
F32 = mybir.dt.float32
I16 = mybir.dt.int16


def _wrap_idx(idx_flat):
    """Lay out a flat int index list for ap_gather: idx[i] must sit at
    partition (i % 16) (replicated across the 8 Q7 groups), column i // 16."""
    n = idx_flat.shape[0]
    assert n % 16 == 0
    w = idx_flat.reshape(n // 16, 16).T.astype(np.int16)   # (16, n//16)
    return np.tile(w, (8, 1))                              # (128, n//16)


def host_prep_image(a_rgb, a_tir):
    """Per-image aux tensors from anchor metadata (all O(N))."""
    aux = {}
    hyhx = []
    for name, a in (("rgb", a_rgb), ("tir", a_tir)):
        ax = a[:, 0].astype(np.float32)
        ay = a[:, 1].astype(np.float32)
        xi = np.floor(ax).astype(np.int32) - 3
        yi = np.floor(ay).astype(np.int32) - 3
        cols = np.clip(xi[:, None] + np.arange(8), 0, W - 1)        # (N,8)
        rows = np.clip(yi[:, None] + np.arange(8), 0, H - 1)        # (N,8)
        gidx = (rows[:, :, None] * W + cols[:, None, :]).reshape(-1)  # (N*64,)
        aux[f"gidx_{name}"] = _wrap_idx(gidx)
        hyhx.append((np.float32(1.0) - (ay - np.floor(ay))).astype(np.float32))
        hyhx.append((np.float32(1.0) - (ax - np.floor(ax))).astype(np.float32))
    # wtab: (128, 512) f32 = [hy_rgb | hx_rgb | hy_tir | hx_tir], replicated rows
    aux["wtab"] = np.tile(np.concatenate(hyhx).astype(np.float32)[None, :], (P, 1))

    # sequential scatter winner map (rgb anchors drive positions)
    ax = a_rgb[:, 0].astype(np.float32)
    ay = a_rgb[:, 1].astype(np.float32)
    x0 = np.clip(np.trunc(ax - np.float32(3.5)).astype(np.int32), 0, W - ROI)
    y0 = np.clip(np.trunc(ay - np.float32(3.5)).astype(np.int32), 0, H - ROI)
    paint = np.full(HW, -1, np.int32)
    for n in range(N):
        for q in range(ROI):
            base = (y0[n] + q) * W + x0[n]
            paint[base:base + ROI] = n * 49 + q * 7 + np.arange(7)
    # per-chunk merge indices into [fused_roi (6272) | chunk_global (2048)]
    minv_cols = []
    for c in range(NCHUNK):
        seg = paint[c * CHUNK_PIX:(c + 1) * CHUNK_PIX]
        inv = np.where(seg >= 0, seg, NPIX + np.arange(CHUNK_PIX))
        minv_cols.append(_wrap_idx(inv))                   # (128, 128)
    aux["minv"] = np.concatenate(minv_cols, axis=1)        # (128, 1024)
    return aux


def host_prep_weights(w_global, b_global, w_att, b_att):
    u_g = (w_global[0] - w_global[1]).astype(np.float32)   # (512,)
    u_a = (w_att[0] - w_att[1]).astype(np.float32)
    # uw: (128, 8): [g_rgb0, g_rgb1, g_tir0, g_tir1, a_rgb0, a_rgb1, a_tir0, a_tir1]
    uw = np.stack([u_g[0:128], u_g[128:256], u_g[256:384], u_g[384:512],
                   u_a[0:128], u_a[128:256], u_a[256:384], u_a[384:512]],
                  axis=1).astype(np.float32)
    c_g = float(np.float32(b_global[0]) - np.float32(b_global[1]))
    c_a = float(np.float32(b_att[0]) - np.float32(b_att[1]))
    return uw, c_g, c_a


def _bcast_w(wt, col_off, n_off, n_cnt, q_cnt, x_cnt):
    """AP over the wtab tile: (128p, n_cnt, q_cnt, x_cnt) reading per-anchor
    weight wtab[p, col_off + n_off + n], broadcast over q/x via 0-strides.
    The trailing broadcast dims must come after the real n dim, so broadcast
    (p, n, q, x) with q/x stride-0, then let the caller transpose if needed."""
    s = wt[:, col_off + n_off:col_off + n_off + n_cnt]
    return s.to_broadcast([P, n_cnt, q_cnt, x_cnt])


def build_program(c_g, c_a):
    nc = bacc.Bacc("TRN2", target_bir_lowering=False, debug=False, num_devices=8)

    fr = nc.dram_tensor("feat_rgb", [C, H, W], F32, kind="ExternalInput")
    ft = nc.dram_tensor("feat_tir", [C, H, W], F32, kind="ExternalInput")
    gidx_r = nc.dram_tensor("gidx_rgb", [P, 512], I16, kind="ExternalInput")
    gidx_t = nc.dram_tensor("gidx_tir", [P, 512], I16, kind="ExternalInput")
    minv = nc.dram_tensor("minv", [P, NCHUNK * 128], I16, kind="ExternalInput")
    wtab = nc.dram_tensor("wtab", [P, 512], F32, kind="ExternalInput")
    uw = nc.dram_tensor("uw", [P, 8], F32, kind="ExternalInput")
    out = nc.dram_tensor("out", [C, H, W], F32, kind="ExternalOutput")

    fr_ap = fr.ap().rearrange("c h w -> c (h w)")
    ft_ap = ft.ap().rearrange("c h w -> c (h w)")
    out_ap = out.ap().rearrange("c h w -> c (h w)")

    with tile.TileContext(nc) as tc, contextlib.ExitStack() as octx:
        persist = octx.enter_context(tc.tile_pool(name="persist", bufs=1))
        gidx_r_sb = persist.tile([P, 512], I16)
        nc.sync.dma_start(out=gidx_r_sb[:], in_=gidx_r.ap())
        gidx_t_sb = persist.tile([P, 512], I16)
        nc.sync.dma_start(out=gidx_t_sb[:], in_=gidx_t.ap())
        wtab_sb = persist.tile([P, 512], F32)
        nc.sync.dma_start(out=wtab_sb[:], in_=wtab.ap())
        uw_sb = persist.tile([P, 8], F32)
        nc.sync.dma_start(out=uw_sb[:], in_=uw.ap())
        ones1 = persist.tile([1, P], F32)
        nc.vector.memset(ones1[:], 1.0)

        # ---------------- Stage A: ROI path ----------------
        R = {}
        with tc.tile_pool(name="rpool", bufs=1, side="right") as rpool:
            with tc.tile_pool(name="gpool", bufs=1, side="right") as gpool, \
                 tc.tile_pool(name="tmpa", bufs=1, side="right") as tmpa, \
                 tc.tile_pool(name="featp", bufs=1, side="right") as featp:
                for ttype, fap, gsb in (("tir", ft_ap, gidx_t_sb),
                                        ("rgb", fr_ap, gidx_r_sb)):
                    wy_off = 0 if ttype == "rgb" else 256
                    wx_off = 128 if ttype == "rgb" else 384
                    for ct in range(CT):
                        fbuf = featp.tile([P, HW], F32, tag="fbuf")
                        nc.sync.dma_start(out=fbuf[:],
                                          in_=fap[ct * P:(ct + 1) * P, :])
                        r = rpool.tile([P, NPIX], F32, tag=f"R{ttype}{ct}",
                                       name=f"R{ttype}{ct}")
                        R[(ttype, ct)] = r
                        r4 = r[:].rearrange("p (n q x) -> p n q x",
                                            n=N, q=ROI, x=ROI)
                        # gather + interp in half-anchor batches of 64
                        for h in range(2):
                            g = gpool.tile([P, 64 * 64], F32, tag="G", bufs=2)
                            if _ABLATE_GATHER:
                                nc.vector.tensor_copy(g[:], fbuf[:, :64 * 64])
                            else:
                                nc.gpsimd.ap_gather(
                                    out_ap=g[:], in_ap=fbuf[:],
                                    idxs_ap=gsb[:, h * 256:(h + 1) * 256],
                                    channels=P, num_elems=HW, d=1,
                                    num_idxs=64 * 64)
                            g4 = g[:].rearrange("p (n q x) -> p n q x",
                                                n=64, q=8, x=8)
                            for qt in range(4):   # 16 anchors per interp chunk
                                ns = slice(qt * 16, (qt + 1) * 16)
                                nabs = h * 64 + qt * 16
                                t1 = tmpa.tile([P, 16, 7, 8], F32, tag="t1")
                                nc.vector.tensor_tensor(
                                    out=t1[:], in0=g4[:, ns, 0:7, :],
                                    in1=g4[:, ns, 1:8, :],
                                    op=mybir.AluOpType.subtract)
                                nc.vector.tensor_tensor(
                                    out=t1[:], in0=t1[:],
                                    in1=_bcast_w(wtab_sb, wy_off, nabs, 16, 7, 8),
                                    op=mybir.AluOpType.mult)
                                yb = tmpa.tile([P, 16, 7, 8], F32, tag="yb")
                                nc.vector.tensor_tensor(
                                    out=yb[:], in0=t1[:], in1=g4[:, ns, 1:8, :],
                                    op=mybir.AluOpType.add)
                                t2 = tmpa.tile([P, 16, 7, 7], F32, tag="t1")
                                nc.vector.tensor_tensor(
                                    out=t2[:], in0=yb[:, :, :, 0:7],
                                    in1=yb[:, :, :, 1:8],
                                    op=mybir.AluOpType.subtract)
                                nc.vector.tensor_tensor(
                                    out=t2[:], in0=t2[:],
                                    in1=_bcast_w(wtab_sb, wx_off, nabs, 16, 7, 7),
                                    op=mybir.AluOpType.mult)
                                nc.vector.tensor_tensor(
                                    out=r4[:, h * 64 + qt * 16:h * 64 + (qt + 1) * 16,
                                           :, :],
                                    in0=t2[:], in1=yb[:, :, :, 1:8],
                                    op=mybir.AluOpType.add)

            # S pool opens once the feat/gather pools are closed; it lives on
            # the LEFT side until the end. The two c-tiles' merge sources are
            # element-interleaved in ONE tile so the merge gather runs once
            # per chunk with d=2 (halves the GPSIMD index walks).
            spool = octx.enter_context(tc.tile_pool(name="spool", bufs=1))
            T = spool.tile([P, SRC_W * 2], F32, name="Til")

            def _til(ct, lo, hi):
                """(128, 1, hi-lo) strided view of interleaved slot ct."""
                return T[:].rearrange("p (s two) -> p two s", two=2)[
                    :, ct:ct + 1, lo:hi]

            # ROI logits (8 anchors / 392 cols per PSUM group), sigmoid, fuse
            with tc.tile_pool(name="fusep", bufs=1, side="right") as fusep, \
                 tc.tile_pool(name="pspa", bufs=1, space="PSUM") as pspa:
                for k in range(16):
                    cs = slice(k * 8 * 49, (k + 1) * 8 * 49)
                    lp = pspa.tile([1, 392], F32, tag="lp", bufs=2)
                    mm = [("rgb", 0, 4), ("rgb", 1, 5),
                          ("tir", 0, 6), ("tir", 1, 7)]
                    for i, (tt, ct, uc) in enumerate(mm):
                        nc.tensor.matmul(
                            out=lp[:], lhsT=uw_sb[:, uc:uc + 1],
                            rhs=R[(tt, ct)][:, cs],
                            start=(i == 0), stop=(i == 3))
                    sig = fusep.tile([1, 392], F32, tag="sig", bufs=2)
                    nc.scalar.activation(
                        out=sig[:], in_=lp[:],
                        func=mybir.ActivationFunctionType.Sigmoid, bias=c_a)
                    sb = pspa.tile([P, 392], F32, tag="sbro", bufs=2)
                    nc.tensor.matmul(out=sb[:], lhsT=ones1[:], rhs=sig[:],
                                     start=True, stop=True)
                    for ct in range(CT):
                        d = fusep.tile([P, 392], F32, tag="fuse", bufs=2)
                        nc.vector.tensor_tensor(
                            out=d[:], in0=R[("rgb", ct)][:, cs],
                            in1=R[("tir", ct)][:, cs],
                            op=mybir.AluOpType.subtract)
                        nc.vector.tensor_tensor(
                            out=d[:], in0=d[:], in1=sb[:],
                            op=mybir.AluOpType.mult)
                        nc.vector.tensor_tensor(
                            out=_til(ct, k * 392, (k + 1) * 392),
                            in0=d[:].unsqueeze(1),
                            in1=R[("tir", ct)][:, cs].unsqueeze(1),
                            op=mybir.AluOpType.add)

        # ---------------- Stage B: global fuse + merge ----------------
        with tc.tile_pool(name="bpool", bufs=1, side="right") as bpool, \
             tc.tile_pool(name="pspb", bufs=1, space="PSUM") as pspb:
            minv_sb = bpool.tile([P, NCHUNK * 128], I16)
            nc.sync.dma_start(out=minv_sb[:], in_=minv.ap())
            for c in range(NCHUNK):
                pix = slice(c * CHUNK_PIX, (c + 1) * CHUNK_PIX)
                ab = {}
                for tt, fap in (("rgb", fr_ap), ("tir", ft_ap)):
                    for ct in range(CT):
                        t = bpool.tile([P, CHUNK_PIX], F32,
                                       tag=f"ch{tt}{ct}", bufs=2,
                                       name=f"ch{tt}{ct}")
                        nc.sync.dma_start(out=t[:],
                                          in_=fap[ct * P:(ct + 1) * P, pix])
                        ab[(tt, ct)] = t
                sgg = bpool.tile([1, CHUNK_PIX], F32, tag="sgg", bufs=2)
                for j in range(4):
                    js = slice(j * 512, (j + 1) * 512)
                    lg = pspb.tile([1, 512], F32, tag="lg", bufs=2)
                    mm = [("rgb", 0, 0), ("rgb", 1, 1),
                          ("tir", 0, 2), ("tir", 1, 3)]
                    for i, (tt, ct, uc) in enumerate(mm):
                        nc.tensor.matmul(
                            out=lg[:], lhsT=uw_sb[:, uc:uc + 1],
                            rhs=ab[(tt, ct)][:, js],
                            start=(i == 0), stop=(i == 3))
                    nc.scalar.activation(
                        out=sgg[:, js], in_=lg[:],
                        func=mybir.ActivationFunctionType.Sigmoid, bias=c_g)
                dbc = pspb.tile([P, CHUNK_PIX], F32, tag="dbc", bufs=1)
                for j in range(4):
                    nc.tensor.matmul(
                        out=dbc[:, j * 512:(j + 1) * 512],
                        lhsT=ones1[:], rhs=sgg[:, j * 512:(j + 1) * 512],
                        start=True, stop=True)
                for ct in range(CT):
                    t = bpool.tile([P, CHUNK_PIX], F32, tag="gbl", bufs=1)
                    nc.vector.tensor_tensor(
                        out=t[:], in0=ab[("rgb", ct)][:], in1=ab[("tir", ct)][:],
                        op=mybir.AluOpType.subtract)
                    nc.vector.tensor_tensor(
                        out=t[:], in0=t[:], in1=dbc[:], op=mybir.AluOpType.mult)
                    nc.vector.tensor_tensor(
                        out=_til(ct, NPIX, SRC_W),
                        in0=t[:].unsqueeze(1),
                        in1=ab[("tir", ct)][:].unsqueeze(1),
                        op=mybir.AluOpType.add)
                # one d=2 gather serves both c-tiles
                finalT = bpool.tile([P, CHUNK_PIX * 2], F32, tag="finT", bufs=2)
                if _ABLATE_GATHER:
                    nc.vector.tensor_copy(finalT[:], T[:, NPIX * 2:])
                else:
                    nc.gpsimd.ap_gather(
                        out_ap=finalT[:], in_ap=T[:],
                        idxs_ap=minv_sb[:, c * 128:(c + 1) * 128],
                        channels=P, num_elems=SRC_W, d=2,
                        num_idxs=CHUNK_PIX)
                fTv = finalT[:].rearrange("p (s two) -> p two s", two=2)
                for ct in range(CT):
                    final = bpool.tile([P, CHUNK_PIX], F32,
                                       tag=f"fin{ct}", bufs=1, name=f"fin{ct}")
                    eng = nc.vector.tensor_copy if ct == 0 else nc.scalar.copy
                    eng(final[:].unsqueeze(1),
                        fTv[:, ct:ct + 1, :])
                    nc.sync.dma_start(
                        out=out_ap[ct * P:(ct + 1) * P, pix], in_=final[:])

    nc.compile()
    return nc


_CACHE = {}


def kernel(feat_rgb, feat_tir, anchors_rgb_with_conf, anchors_tir_with_conf,
           w_global, b_global, w_att, b_att):
    feat_rgb = np.asarray(feat_rgb, dtype=np.float32)
    feat_tir = np.asarray(feat_tir, dtype=np.float32)
    a_rgb = np.asarray(anchors_rgb_with_conf, dtype=np.float32)
    a_tir = np.asarray(anchors_tir_with_conf, dtype=np.float32)
    w_global = np.asarray(w_global, dtype=np.float32)
    b_global = np.asarray(b_global, dtype=np.float32)
    w_att = np.asarray(w_att, dtype=np.float32)
    b_att = np.asarray(b_att, dtype=np.float32)

    B = feat_rgb.shape[0]
    assert B == 8

    uw, c_g, c_a = host_prep_weights(w_global, b_global, w_att, b_att)
    key = (c_g, c_a)
    if key not in _CACHE:
        _CACHE[key] = build_program(c_g, c_a)
    nc = _CACHE[key]

    in_maps = []
    for b in range(B):
        aux = host_prep_image(a_rgb[b], a_tir[b])
        in_maps.append({
            "feat_rgb": np.ascontiguousarray(feat_rgb[b]),
            "feat_tir": np.ascontiguousarray(feat_tir[b]),
            "gidx_rgb": aux["gidx_rgb"],
            "gidx_tir": aux["gidx_tir"],
            "minv": aux["minv"],
            "wtab": aux["wtab"],
            "uw": uw,
        })

    res = run_bass_kernel_spmd(nc, in_maps, core_ids=list(range(8)))
    global LAST_RUN
    LAST_RUN = res
    outs = [res.results[b]["out"] for b in range(B)]
    return np.stack(outs).astype(np.float32)


LAST_RUN = None


def time_kernel_ns(feat_rgb, feat_tir, anchors_rgb_with_conf,
                   anchors_tir_with_conf, w_global, b_global, w_att, b_att,
                   inner_iters=8, outer_iters=3):
    """Best-effort HW execution time: run the compiled NEFF `inner_iters`
    times inside one jitted call (serialized through the donated output
    buffers so XLA cannot dedupe), amortizing the axon dispatch overhead.
    Returns ns per NEFF execution (min over outer_iters)."""
    import time as _time
    import jax
    import jax.numpy as jnp
    from jax.sharding import Mesh, PartitionSpec
    from jax.experimental.shard_map import shard_map
    from concourse import bass2jax

    feat_rgb = np.asarray(feat_rgb, dtype=np.float32)
    feat_tir = np.asarray(feat_tir, dtype=np.float32)
    a_rgb = np.asarray(anchors_rgb_with_conf, dtype=np.float32)
    a_tir = np.asarray(anchors_tir_with_conf, dtype=np.float32)
    uw, c_g, c_a = host_prep_weights(
        np.asarray(w_global, np.float32), np.asarray(b_global, np.float32),
        np.asarray(w_att, np.float32), np.asarray(b_att, np.float32))
    key = (c_g, c_a)
    if key not in _CACHE:
        _CACHE[key] = build_program(c_g, c_a)
    nc = _CACHE[key]
    bass2jax.install_neuronx_cc_hook()

    in_names = []
    out_names = []
    out_avals = []
    partition_name = nc.partition_id_tensor.name if nc.partition_id_tensor else None
    for alloc in nc.m.functions[0].allocations:
        import concourse.mybir as mybir_
        if not isinstance(alloc, mybir_.MemoryLocationSet):
            continue
        name = alloc.memorylocations[0].name
        if alloc.kind == "ExternalInput":
            if name != partition_name:
                in_names.append(name)
        elif alloc.kind == "ExternalOutput":
            out_names.append(name)
            out_avals.append(jax.core.ShapedArray(
                tuple(alloc.tensor_shape), mybir.dt.np(alloc.dtype)))
    n_params = len(in_names)
    all_names = list(in_names) + list(out_names)
    if partition_name is not None:
        all_names.append(partition_name)

    def _body(*args):
        operands = list(args)
        if partition_name is not None:
            operands.append(bass2jax.partition_id_tensor())
        outs = bass2jax._bass_exec_p.bind(
            *operands, out_avals=tuple(out_avals), in_names=tuple(all_names),
            out_names=tuple(out_names), lowering_input_output_aliases=(),
            sim_require_finite=True, sim_require_nnan=True, nc=nc)
        return tuple(outs)

    del inner_iters  # the compile hook allows one bass_exec per module
    _loop = _body

    in_maps = []
    B = feat_rgb.shape[0]
    for b in range(B):
        aux = host_prep_image(a_rgb[b], a_tir[b])
        in_maps.append({
            "feat_rgb": np.ascontiguousarray(feat_rgb[b]),
            "feat_tir": np.ascontiguousarray(feat_tir[b]),
            "gidx_rgb": aux["gidx_rgb"], "gidx_tir": aux["gidx_tir"],
            "minv": aux["minv"], "wtab": aux["wtab"], "uw": uw,
        })

    devices = jax.devices()[:B]
    mesh = Mesh(np.asarray(devices), ("core",))
    n_outs = len(out_names)
    sharded = jax.jit(
        shard_map(_loop, mesh=mesh,
                  in_specs=(PartitionSpec("core"),) * (n_params + n_outs),
                  out_specs=(PartitionSpec("core"),) * n_outs,
                  check_rep=False),
        keep_unused=True)
    concat_in = [np.concatenate([np.asarray(in_maps[c][nm]) for c in range(B)],
                                axis=0) for nm in in_names]
    concat_zeros = [np.zeros((B * a.shape[0], *a.shape[1:]), a.dtype)
                    for a in out_avals]
    from jax.sharding import NamedSharding
    shard = NamedSharding(mesh, PartitionSpec("core"))
    dev_in = [jax.device_put(x, shard) for x in concat_in]
    dev_zeros = [jax.device_put(z, shard) for z in concat_zeros]
    # warmup (compiles)
    jax.block_until_ready(sharded(*dev_in, *dev_zeros))

    def run_k(k):
        best = None
        for _ in range(outer_iters):
            t0 = _time.perf_counter()
            outs = None
            for _ in range(k):
                outs = sharded(*dev_in, *dev_zeros)
            jax.block_until_ready(outs)
            dt = _time.perf_counter() - t0
            best = dt if best is None else min(best, dt)
        return best

    t1 = run_k(1)
    t9 = run_k(9)
    per_exec = (t9 - t1) / 8
    print(f"  (1 call {t1*1e6:.0f} us, 9 calls {t9*1e6:.0f} us)")
    return max(per_exec, 0.0) * 1e9


if __name__ == "__main__":
    import reference
    inputs = reference.setup_inputs()
    actual = kernel(**{k: np.asarray(v) for k, v in inputs.items()})
    expected = np.asarray(reference.reference(**inputs))
    err = np.abs(actual - expected).max()
    print("abs err:", err, "rel:", err / np.abs(expected).max())
